# revision 1
# baseline (speedup 1.0000x reference)
"""KNRM ranking kernel, data-parallel over batch across 8 NeuronCores.

Strategy (per spec sharding_hint): shard B=1024 into 8 x 128 rows; replicate
the embedding table and MLP weights on every core. Each core gathers its
shard's q/d embeddings, L2-normalizes, builds the cosine matching matrix,
applies the 21 RBF kernels + log1p pooling, runs the 3-layer MLP for both
(q1,d1) and (q2,d2), and emits sigmoid(l1 - l2) for its 128 rows.
"""

import numpy as np
import jax
import jax.numpy as jnp
from functools import partial

# ---- hardcoded problem constants (from the KNRM spec) ----
B, Q, D, E = 1024, 64, 256, 128
VOCAB = 100000
N_CORES = 8
KERNEL_NUM, SIGMA, EXACT_SIGMA = 21, 0.1, 0.001


def _mus_sigmas():
    step = 2.0 / (KERNEL_NUM - 1)
    mus = [1.0, 1.0 - step / 2]
    for i in range(1, KERNEL_NUM - 1):
        mus.append(mus[i] - step)
    mus = list(reversed(mus))
    sigmas = [SIGMA] * (KERNEL_NUM - 1) + [EXACT_SIGMA]
    return np.array(mus, np.float32), np.array(sigmas, np.float32)


MUS, SIGMAS = _mus_sigmas()


def _predict(emb, q_ids, d_ids, W1, b1, W2, b2, W3, b3):
    # gather + normalize
    q = emb[q_ids]  # [b,Q,E]
    d = emb[d_ids]  # [b,D,E]
    qn = q / (jnp.linalg.norm(q, axis=-1, keepdims=True) + 1e-13)
    dn = d / (jnp.linalg.norm(d, axis=-1, keepdims=True) + 1e-13)
    M = jnp.einsum("bqe,bde->bqd", qn, dn)  # [b,Q,D]
    # kernel pooling
    mus = jnp.asarray(MUS)[None, None, None, :]  # [1,1,1,21]
    sigs = jnp.asarray(SIGMAS)[None, None, None, :]
    k = jnp.exp(-((M[..., None] - mus) ** 2) / (2.0 * sigs**2))  # [b,Q,D,21]
    ko = jnp.log1p(k.sum(axis=2)).sum(axis=1)  # [b,21]
    # MLP (ReLU before each linear)
    x = jax.nn.relu(ko) @ W1 + b1
    x = jax.nn.relu(x) @ W2 + b2
    x = jax.nn.relu(x) @ W3 + b3
    return x  # [b,1]


@partial(
    jax.pmap,
    axis_name="x",
    in_axes=(0, 0, 0, 0, None, None, None, None, None, None, None),
)
def _shard_fn(q1, d1, q2, d2, emb, W1, b1, W2, b2, W3, b3):
    l1 = _predict(emb, q1, d1, W1, b1, W2, b2, W3, b3)
    l2 = _predict(emb, q2, d2, W1, b1, W2, b2, W3, b3)
    return jax.nn.sigmoid(l1 - l2)


def kernel(q1, d1, q2, d2, emb, W1, b1, W2, b2, W3, b3):
    shard = B // N_CORES  # 128
    q1s = np.asarray(q1).reshape(N_CORES, shard, Q)
    d1s = np.asarray(d1).reshape(N_CORES, shard, D)
    q2s = np.asarray(q2).reshape(N_CORES, shard, Q)
    d2s = np.asarray(d2).reshape(N_CORES, shard, D)
    out = _shard_fn(
        q1s, d1s, q2s, d2s,
        jnp.asarray(emb), jnp.asarray(W1), jnp.asarray(b1),
        jnp.asarray(W2), jnp.asarray(b2), jnp.asarray(W3), jnp.asarray(b3),
    )
    return np.asarray(out).reshape(B, 1).astype(np.float32)

